# revision 41
# baseline (speedup 1.0000x reference)
"""Trainium2 Bass kernel for nn_MessageFunction (gnn_message_passing).

Math (validated against the reference):
  The reference broadcasts h_w[:, :, None] -> (B*N, IN_F, N) and reshapes to
  [E, IN_F]; row-major order makes every row constant:
      h_w_rows[e, i] = h_w.reshape(-1)[e]   for all i.
  Hence the per-edge bmm collapses:
      m[e, o] = sum_i edge_output[e, o, i] * s[e]
              = s[e] * (x3[e] @ W4s[:, o] + b4s[o])
  with W4s = W4.reshape(HID3, OUT_F, IN_F).sum(-1), b4s = b4.reshape(OUT_F,
  IN_F).sum(-1), s = h_w.reshape(-1).  This is an exact reassociation (only
  f32 rounding differences) and removes the [E,128]@[128,4096] matmul + bmm.

Kernel: data-parallel over E = 32768 edges, 4096 per core across 8 cores,
MLP weights replicated, no cross-core communication.  Per core the MLP runs
features-on-partitions with edges streaming on the free dim:
    x1 = relu(W1.T @ eT)        K=32  -> [128, e]
    x2 = relu(W2.T @ x1)        K=128 -> [256, e] (two 128-part halves)
    x3 = relu(W3.T @ x2)        K=256 -> [128, e] (PSUM accumulation)
    y  = W4s.T @ x3             K=128 -> [64, e]  (col-packed 2 tiles/PSUM)
    out = (y + b4s) * s         one fused scalar_tensor_tensor on VectorE
Matmuls use float32r (full PE rate at N=512, near-fp32 precision).
"""

import os

import numpy as np

import concourse.bacc as bacc
import concourse.bass as bass
import concourse.mybir as mybir
import concourse.tile as tile
from concourse.bass_utils import run_bass_kernel_spmd

# Problem constants (hardcoded per the harness contract).
B, N = 8, 64
IN_F, OUT_F = 64, 64
EDGE_F = 32
HID1, HID2, HID3 = 128, 256, 128
E = B * N * N            # 32768
N_CORES = 8
E_LOC = E // N_CORES     # 4096
TILE = 512               # edges per tile (one PSUM bank per stage)
NT = E_LOC // TILE       # 8 tiles per core
OUT_CHUNK = 1024         # output DMA granularity (2 tiles)

F32 = mybir.dt.float32
# Matmul operand dtype: float32r streams at 1 cycle/row for N>=256 (same as
# bf16) with much better precision than bf16.
DT = mybir.dt.float32r
NP_DT = np.float32

# Module global: last BassKernelResults (test.py reads exec_time_ns from it).
LAST_RESULTS = None


def _build_bass(b2_halves_equal=True):
    nc = bacc.Bacc(
        "TRN2", target_bir_lowering=False, debug=False, num_devices=N_CORES
    )

    # Per-core inputs.  e_t is packed 4 edge-groups deep on partitions:
    # e_t[32*g + f, c] = e_vw[g*1024 + c, f] so one full-width DMA loads it.
    e_t = nc.dram_tensor("e_t", [128, E_LOC // 4], DT, kind="ExternalInput")
    s_b = nc.dram_tensor("s_b", [OUT_F, E_LOC], F32, kind="ExternalInput")
    # Replicated weights.  W1 is stacked 4x on partitions to serve the four
    # L1 row-tile positions.
    w1d = nc.dram_tensor("w1d", [128, HID1], DT, kind="ExternalInput")
    # Everything else packed into one DMA: [W2 | W3packed | W4s | biases]
    # = 256 + 256 + 64 + 5 columns.  Bias columns: b1, b2[:128], b2[128:],
    # b3, [b4s; pad].
    WPACK = HID2 + 2 * HID3 + OUT_F + 5
    wpd = nc.dram_tensor("wpd", [128, WPACK], DT, kind="ExternalInput")
    outd = nc.dram_tensor(
        "outd", [E_LOC // OUT_CHUNK, OUT_F, OUT_CHUNK], F32, kind="ExternalOutput"
    )

    # Relu pass engine schedule (per tile: L1, L2a, L2b, L3). 'A' = ScalarE,
    # 'V' = VectorE.  VectorE also runs the eight final bias+scale ops, so
    # ScalarE takes more of the 32 relu passes (20 A / 12 V).
    relu_sched = ["AVAV", "AVAA"] * (NT // 2)

    with tile.TileContext(nc) as tc:
        with (
            tc.tile_pool(name="wp", bufs=1) as wp,
            tc.tile_pool(name="io", bufs=4) as io,
            tc.tile_pool(name="acts", bufs=3) as acts,
            tc.tile_pool(name="ps", bufs=1, space="PSUM") as ps,
        ):
            e4 = wp.tile([128, E_LOC // 4], DT, tag="e4")
            w1 = wp.tile([128, HID1], DT, tag="w1")
            wpk = wp.tile([128, WPACK], DT, tag="wpk")
            s_sb = wp.tile([OUT_F, E_LOC], F32, tag="s_sb")
            out_sb = wp.tile([OUT_F, E_LOC], F32, tag="out_sb")
            # Views into the packed weight tile (bias columns bitcast to f32;
            # both dtypes are 4-byte so this is a pure reinterpret).
            w2 = wpk[:, 0:HID2]
            w3 = wpk[:, HID2 : HID2 + 2 * HID3]
            w4 = wpk[:, HID2 + 2 * HID3 : HID2 + 2 * HID3 + OUT_F]
            bb = wpk[:, HID2 + 2 * HID3 + OUT_F :].bitcast(F32)
            # Input loads on the two HWDGE rings (Sync + Scalar), which issue
            # in ~0.6us and stream FIFO; the GpSimd SWDGE path costs ~2us
            # fixed per transfer, far too slow for the startup path.
            # Sync ring carries ONLY what the first L1 matmul needs (ring is
            # FIFO end-to-end, so anything else here delays the whole kernel).
            nc.sync.dma_start(w1[:], w1d[:])
            nc.sync.dma_start(e4[:], e_t[:])
            nc.scalar.dma_start(wpk[:], wpd[:])
            nc.scalar.dma_start(s_sb[:], s_b[:])

            # PE warm-up: ~10 dependency-free matmuls on scratch data run
            # back-to-back during the input-load window, so the HAM clock
            # gate reaches 2.4 GHz before the first real matmul.  Garbage
            # values are fine — the scratch PSUM is never read.
            scratch = wp.tile([128, TILE], DT, tag="scratch")
            nc.gpsimd.memset(scratch[:].bitcast(F32), 1.0)

            def emit_dummies(n):
                for _ in range(n):
                    warm_ps = ps.tile([OUT_F, TILE], F32, tag="y4p", bufs=2)
                    nc.tensor.matmul(warm_ps[:], scratch[:, 0:OUT_F], scratch[:])

            emit_dummies(10)

            def relu_pass(dst, src, bias_col, eng):
                if eng == "A":
                    nc.scalar.activation(
                        dst, src, mybir.ActivationFunctionType.Relu, bias=bias_col
                    )
                else:
                    nc.vector.tensor_scalar(
                        out=dst,
                        in0=src,
                        scalar1=bias_col,
                        scalar2=0.0,
                        op0=mybir.AluOpType.add,
                        op1=mybir.AluOpType.max,
                    )

            # Software-pipelined emission, skewed so each pass result is
            # consumed one full iteration after it is produced — the PE
            # matmul stream never waits on a just-issued ScalarE/VectorE
            # pass.  Stage s of tile t runs in iteration t+s.
            x1_t = [None] * NT
            x2_t = [None] * NT
            x3_t = [None] * NT
            # P3 engine: 6 on ScalarE / 2 on VectorE (balance against the
            # fixed STT work on VectorE); P1 on VectorE, P2 on ScalarE.
            p3_eng = ["A", "A", "V", "A", "A", "A", "V", "A"]

            # Dummy matmuls bridging the pipeline-fill iterations, so the PE
            # never idles >3.4us (which would re-throttle the HAM clock gate).
            bridge = {0: 4, 1: 3, 2: 2}

            for i in range(NT + 4):
                if i in bridge:
                    emit_dummies(bridge[i])
                # S1 + P1 for tile i.  L1 is a K=32 row-tiled matmul: edge
                # group g = i//2 lives on partitions [32g, 32g+32) of e4 and
                # w1 (stacked), with the matching tile_position row.
                if 0 <= i < NT:
                    g = i // 2
                    gp = slice(32 * g, 32 * g + 32)
                    gc = slice((i % 2) * TILE, (i % 2) * TILE + TILE)
                    x1p = ps.tile([128, TILE], F32, tag="x1p", bufs=2)
                    nc.tensor.matmul(
                        x1p[:], w1[gp, :], e4[gp, gc], tile_position=(32 * g, 0)
                    )
                    x1 = acts.tile([128, TILE], DT, tag="x1")
                    relu_pass(x1[:], x1p[:], bb[:, 0:1], "V")
                    x1_t[i] = x1

                # S2 + P2 for tile i-1 (merged 2-bank PSUM, single pass)
                j = i - 1
                if 0 <= j < NT:
                    x2p = ps.tile([128, 2 * TILE], F32, tag="x2p", bufs=1)
                    nc.tensor.matmul(x2p[:, 0:TILE], w2[:, 0:128], x1_t[j][:])
                    nc.tensor.matmul(x2p[:, TILE : 2 * TILE], w2[:, 128:256], x1_t[j][:])
                    x2 = acts.tile([128, 2 * TILE], DT, tag="x2")
                    # A per-partition bias is constant along the free dim, so
                    # one merged pass is only valid when both b2 halves agree
                    # (always true for the zero biases here); otherwise fall
                    # back to two passes.
                    if b2_halves_equal:
                        nc.scalar.activation(
                            x2[:], x2p[:],
                            mybir.ActivationFunctionType.Relu, bias=bb[:, 1:2],
                        )
                    else:
                        nc.scalar.activation(
                            x2[:, 0:TILE], x2p[:, 0:TILE],
                            mybir.ActivationFunctionType.Relu, bias=bb[:, 1:2],
                        )
                        nc.scalar.activation(
                            x2[:, TILE : 2 * TILE], x2p[:, TILE : 2 * TILE],
                            mybir.ActivationFunctionType.Relu, bias=bb[:, 2:3],
                        )
                    x2_t[j] = x2
                    x1_t[j] = None

                # S3 + P3 for tile i-2
                j = i - 2
                if 0 <= j < NT:
                    x3ps = ps.tile([128, TILE], F32, tag="x3ps", bufs=2)
                    nc.tensor.matmul(
                        x3ps[:], w3[:, 0:128], x2_t[j][:, 0:TILE],
                        start=True, stop=False,
                    )
                    nc.tensor.matmul(
                        x3ps[:], w3[:, 128:256], x2_t[j][:, TILE : 2 * TILE],
                        start=False, stop=True,
                    )
                    x3 = acts.tile([128, TILE], DT, tag="x3")
                    relu_pass(x3[:], x3ps[:], bb[:, 3:4], p3_eng[j])
                    x3_t[j] = x3
                    x2_t[j] = None

                # S4 + P4 for tile i-3
                j = i - 3
                if 0 <= j < NT:
                    cs = slice(j * TILE, (j + 1) * TILE)
                    y4p = ps.tile([OUT_F, TILE], F32, tag="y4p", bufs=2)
                    nc.tensor.matmul(y4p[:], w4[:], x3_t[j][:])
                    nc.vector.scalar_tensor_tensor(
                        out=out_sb[:, cs],
                        in0=y4p[:],
                        scalar=bb[0:OUT_F, 4:5],
                        in1=s_sb[:, cs],
                        op0=mybir.AluOpType.add,
                        op1=mybir.AluOpType.mult,
                    )
                    x3_t[j] = None
                    if (j + 1) * TILE % OUT_CHUNK == 0:
                        ck = ((j + 1) * TILE) // OUT_CHUNK - 1
                        nc.sync.dma_start(
                            outd[ck],
                            out_sb[:, ck * OUT_CHUNK : (ck + 1) * OUT_CHUNK],
                        )

    nc.compile()
    return nc


_CACHED_NC = None


def kernel(h_v, h_w, e_vw, W1, b1, W2, b2, W3, b3, W4, b4):
    global LAST_RESULTS, _CACHED_NC

    h_w = np.asarray(h_w, np.float32)
    e_vw = np.asarray(e_vw, np.float32)
    W1 = np.asarray(W1, np.float32)
    W2 = np.asarray(W2, np.float32)
    W3 = np.asarray(W3, np.float32)
    W4 = np.asarray(W4, np.float32)
    b1 = np.asarray(b1, np.float32)
    b2 = np.asarray(b2, np.float32)
    b3 = np.asarray(b3, np.float32)
    b4 = np.asarray(b4, np.float32)

    # Host-side weight transform (exact reassociation of the reference math).
    W4s = W4.reshape(HID3, OUT_F, IN_F).sum(axis=2)
    b4s = b4.reshape(OUT_F, IN_F).sum(axis=1)
    s = h_w.reshape(-1)

    w3p = np.concatenate([W3[0:128], W3[128:256]], axis=1)  # [128, 256]
    bb = np.zeros((128, 5), np.float32)
    bb[:, 0] = b1
    bb[:, 1] = b2[0:128]
    bb[:, 2] = b2[128:256]
    bb[:, 3] = b3
    bb[0:OUT_F, 4] = b4s

    wpack = np.concatenate([W2, w3p, W4s, bb], axis=1)  # [128, 581]
    weights_map = {
        "w1d": np.ascontiguousarray(np.tile(W1, (4, 1)), NP_DT),
        "wpd": np.ascontiguousarray(wpack, np.float32),
    }

    in_maps = []
    for c in range(N_CORES):
        sl = slice(c * E_LOC, (c + 1) * E_LOC)
        e_loc = e_vw[sl]                       # [4096, 32]
        s_loc = s[sl]                          # [4096]
        # [128, 1024]: partition 32g+f holds feature f of edge group g
        e_t = np.ascontiguousarray(
            e_loc.T.reshape(EDGE_F, 4, E_LOC // 4)
            .transpose(1, 0, 2)
            .reshape(128, E_LOC // 4),
            NP_DT,
        )
        s_bcast = np.ascontiguousarray(
            np.broadcast_to(s_loc[None, :], (OUT_F, E_LOC)), np.float32
        )
        in_maps.append({"e_t": e_t, "s_b": s_bcast, **weights_map})

    if _CACHED_NC is None:
        _CACHED_NC = _build_bass(
            b2_halves_equal=bool(np.array_equal(b2[0:128], b2[128:256]))
        )
    nc = _CACHED_NC

    trace = bool(int(os.environ.get("KERNEL_TRACE", "0")))
    res = run_bass_kernel_spmd(
        nc, in_maps, core_ids=list(range(N_CORES)), trace=trace
    )
    LAST_RESULTS = res

    out = np.empty((E, OUT_F), np.float32)
    nck = E_LOC // OUT_CHUNK
    for c in range(N_CORES):
        o = res.results[c]["outd"]             # [nck, OUT_F, OUT_CHUNK]
        base = c * E_LOC
        for k in range(nck):
            out[base + k * OUT_CHUNK : base + (k + 1) * OUT_CHUNK] = o[k].T
    return out


# revision 42
# speedup vs baseline: 1.2930x; 1.2930x over previous
"""Trainium2 Bass kernel for nn_MessageFunction (gnn_message_passing).

Math (validated against the reference):
  The reference broadcasts h_w[:, :, None] -> (B*N, IN_F, N) and reshapes to
  [E, IN_F]; row-major order makes every row constant:
      h_w_rows[e, i] = h_w.reshape(-1)[e]   for all i.
  Hence the per-edge bmm collapses:
      m[e, o] = sum_i edge_output[e, o, i] * s[e]
              = s[e] * (x3[e] @ W4s[:, o] + b4s[o])
  with W4s = W4.reshape(HID3, OUT_F, IN_F).sum(-1), b4s = b4.reshape(OUT_F,
  IN_F).sum(-1), s = h_w.reshape(-1).  This is an exact reassociation (only
  f32 rounding differences) and removes the [E,128]@[128,4096] matmul + bmm.

Kernel: data-parallel over E = 32768 edges, 4096 per core across 8 cores,
MLP weights replicated, no cross-core communication.  Per core the MLP runs
features-on-partitions with edges streaming on the free dim:
    x1 = relu(W1.T @ eT)        K=32  -> [128, e]
    x2 = relu(W2.T @ x1)        K=128 -> [256, e] (two 128-part halves)
    x3 = relu(W3.T @ x2)        K=256 -> [128, e] (PSUM accumulation)
    y  = W4s.T @ x3             K=128 -> [64, e]  (col-packed 2 tiles/PSUM)
    out = (y + b4s) * s         one fused scalar_tensor_tensor on VectorE
Matmuls use float32r (full PE rate at N=512, near-fp32 precision).
"""

import os

import numpy as np

import concourse.bacc as bacc
import concourse.bass as bass
import concourse.mybir as mybir
import concourse.tile as tile
from concourse.bass_utils import run_bass_kernel_spmd

# Problem constants (hardcoded per the harness contract).
B, N = 8, 64
IN_F, OUT_F = 64, 64
EDGE_F = 32
HID1, HID2, HID3 = 128, 256, 128
E = B * N * N            # 32768
N_CORES = 8
E_LOC = E // N_CORES     # 4096
TILE = 512               # edges per tile (one PSUM bank per stage)
NT = E_LOC // TILE       # 8 tiles per core
OUT_CHUNK = 1024         # output DMA granularity (2 tiles)

F32 = mybir.dt.float32
# Matmul operand dtype: float32r streams at 1 cycle/row for N>=256 (same as
# bf16) with much better precision than bf16.
DT = mybir.dt.float32r
NP_DT = np.float32

# Module global: last BassKernelResults (test.py reads exec_time_ns from it).
LAST_RESULTS = None


def _build_bass(b2_halves_equal=True):
    nc = bacc.Bacc(
        "TRN2", target_bir_lowering=False, debug=False, num_devices=N_CORES
    )

    # Per-core inputs.  e_t is packed 4 edge-groups deep on partitions:
    # e_t[32*g + f, c] = e_vw[g*1024 + c, f] so one full-width DMA loads it.
    e_t = nc.dram_tensor("e_t", [128, E_LOC // 4], DT, kind="ExternalInput")
    s_b = nc.dram_tensor("s_b", [OUT_F, E_LOC], F32, kind="ExternalInput")
    # Replicated weights.  W1 is stacked 4x on partitions to serve the four
    # L1 row-tile positions.
    w1d = nc.dram_tensor("w1d", [128, HID1], DT, kind="ExternalInput")
    # Everything else packed into one DMA: [W2 | W3packed | W4s | biases]
    # = 256 + 256 + 64 + 5 columns.  Bias columns: b1, b2[:128], b2[128:],
    # b3, [b4s; pad].
    WPACK = HID2 + 2 * HID3 + OUT_F + 5
    wpd = nc.dram_tensor("wpd", [128, WPACK], DT, kind="ExternalInput")
    outd = nc.dram_tensor(
        "outd", [E_LOC // OUT_CHUNK, OUT_F, OUT_CHUNK], F32, kind="ExternalOutput"
    )

    # Relu pass engine schedule (per tile: L1, L2a, L2b, L3). 'A' = ScalarE,
    # 'V' = VectorE.  VectorE also runs the eight final bias+scale ops, so
    # ScalarE takes more of the 32 relu passes (20 A / 12 V).
    relu_sched = ["AVAV", "AVAA"] * (NT // 2)

    with tile.TileContext(nc) as tc:
        with (
            tc.tile_pool(name="wp", bufs=1) as wp,
            tc.tile_pool(name="io", bufs=4) as io,
            tc.tile_pool(name="acts", bufs=3) as acts,
            tc.tile_pool(name="ps", bufs=1, space="PSUM") as ps,
        ):
            e4 = wp.tile([128, E_LOC // 4], DT, tag="e4")
            w1 = wp.tile([128, HID1], DT, tag="w1")
            wpk = wp.tile([128, WPACK], DT, tag="wpk")
            s_sb = wp.tile([OUT_F, E_LOC], F32, tag="s_sb")
            out_sb = wp.tile([OUT_F, E_LOC], F32, tag="out_sb")
            # Views into the packed weight tile (bias columns bitcast to f32;
            # both dtypes are 4-byte so this is a pure reinterpret).
            w2 = wpk[:, 0:HID2]
            w3 = wpk[:, HID2 : HID2 + 2 * HID3]
            w4 = wpk[:, HID2 + 2 * HID3 : HID2 + 2 * HID3 + OUT_F]
            bb = wpk[:, HID2 + 2 * HID3 + OUT_F :].bitcast(F32)
            # Input loads on the two HWDGE rings (Sync + Scalar), which issue
            # in ~0.6us and stream FIFO; the GpSimd SWDGE path costs ~2us
            # fixed per transfer, far too slow for the startup path.
            # Sync ring carries ONLY what the first L1 matmul needs (ring is
            # FIFO end-to-end, so anything else here delays the whole kernel).
            nc.sync.dma_start(w1[:], w1d[:])
            nc.sync.dma_start(e4[:], e_t[:])
            nc.scalar.dma_start(wpk[:], wpd[:])
            nc.scalar.dma_start(s_sb[:], s_b[:])

            # PE warm-up: ~10 dependency-free matmuls on scratch data run
            # back-to-back during the input-load window, so the HAM clock
            # gate reaches 2.4 GHz before the first real matmul.  Garbage
            # values are fine — the scratch PSUM is never read.
            scratch = wp.tile([128, TILE], DT, tag="scratch")
            nc.gpsimd.memset(scratch[:].bitcast(F32), 1.0)

            def emit_dummies(n):
                for _ in range(n):
                    warm_ps = ps.tile([OUT_F, TILE], F32, tag="y4p", bufs=2)
                    nc.tensor.matmul(warm_ps[:], scratch[:, 0:OUT_F], scratch[:])

            emit_dummies(10)

            def relu_pass(dst, src, bias_col, eng):
                if eng == "A":
                    nc.scalar.activation(
                        dst, src, mybir.ActivationFunctionType.Relu, bias=bias_col
                    )
                else:
                    nc.vector.tensor_scalar(
                        out=dst,
                        in0=src,
                        scalar1=bias_col,
                        scalar2=0.0,
                        op0=mybir.AluOpType.add,
                        op1=mybir.AluOpType.max,
                    )

            # Software-pipelined emission, skewed so each pass result is
            # consumed one full iteration after it is produced — the PE
            # matmul stream never waits on a just-issued ScalarE/VectorE
            # pass.  Stage s of tile t runs in iteration t+s.
            x1_t = [None] * NT
            x2_t = [None] * NT
            x3_t = [None] * NT
            # P3 engine: 6 on ScalarE / 2 on VectorE (balance against the
            # fixed STT work on VectorE); P1 on VectorE, P2 on ScalarE.
            p3_eng = ["A", "A", "V", "A", "A", "A", "V", "A"]

            # Dummy matmuls bridging the pipeline-fill iterations, so the PE
            # never idles >3.4us (which would re-throttle the HAM clock gate).
            bridge = {}

            for i in range(NT + 4):
                if i in bridge:
                    emit_dummies(bridge[i])
                # S1 + P1 for tile i.  L1 is a K=32 row-tiled matmul: edge
                # group g = i//2 lives on partitions [32g, 32g+32) of e4 and
                # w1 (stacked), with the matching tile_position row.
                if 0 <= i < NT:
                    g = i // 2
                    gp = slice(32 * g, 32 * g + 32)
                    gc = slice((i % 2) * TILE, (i % 2) * TILE + TILE)
                    x1p = ps.tile([128, TILE], F32, tag="x1p", bufs=2)
                    nc.tensor.matmul(
                        x1p[:], w1[gp, :], e4[gp, gc], tile_position=(32 * g, 0)
                    )
                    x1 = acts.tile([128, TILE], DT, tag="x1")
                    relu_pass(x1[:], x1p[:], bb[:, 0:1], "V")
                    x1_t[i] = x1

                # S2 + P2 for tile i-1 (merged 2-bank PSUM, single pass)
                j = i - 1
                if 0 <= j < NT:
                    x2p = ps.tile([128, 2 * TILE], F32, tag="x2p", bufs=1)
                    nc.tensor.matmul(x2p[:, 0:TILE], w2[:, 0:128], x1_t[j][:])
                    nc.tensor.matmul(x2p[:, TILE : 2 * TILE], w2[:, 128:256], x1_t[j][:])
                    x2 = acts.tile([128, 2 * TILE], DT, tag="x2")
                    # A per-partition bias is constant along the free dim, so
                    # one merged pass is only valid when both b2 halves agree
                    # (always true for the zero biases here); otherwise fall
                    # back to two passes.
                    if b2_halves_equal:
                        nc.scalar.activation(
                            x2[:], x2p[:],
                            mybir.ActivationFunctionType.Relu, bias=bb[:, 1:2],
                        )
                    else:
                        nc.scalar.activation(
                            x2[:, 0:TILE], x2p[:, 0:TILE],
                            mybir.ActivationFunctionType.Relu, bias=bb[:, 1:2],
                        )
                        nc.scalar.activation(
                            x2[:, TILE : 2 * TILE], x2p[:, TILE : 2 * TILE],
                            mybir.ActivationFunctionType.Relu, bias=bb[:, 2:3],
                        )
                    x2_t[j] = x2
                    x1_t[j] = None

                # S3 + P3 for tile i-2
                j = i - 2
                if 0 <= j < NT:
                    x3ps = ps.tile([128, TILE], F32, tag="x3ps", bufs=2)
                    nc.tensor.matmul(
                        x3ps[:], w3[:, 0:128], x2_t[j][:, 0:TILE],
                        start=True, stop=False,
                    )
                    nc.tensor.matmul(
                        x3ps[:], w3[:, 128:256], x2_t[j][:, TILE : 2 * TILE],
                        start=False, stop=True,
                    )
                    x3 = acts.tile([128, TILE], DT, tag="x3")
                    relu_pass(x3[:], x3ps[:], bb[:, 3:4], p3_eng[j])
                    x3_t[j] = x3
                    x2_t[j] = None

                # S4 + P4 for tile i-3
                j = i - 3
                if 0 <= j < NT:
                    cs = slice(j * TILE, (j + 1) * TILE)
                    y4p = ps.tile([OUT_F, TILE], F32, tag="y4p", bufs=2)
                    nc.tensor.matmul(y4p[:], w4[:], x3_t[j][:])
                    nc.vector.scalar_tensor_tensor(
                        out=out_sb[:, cs],
                        in0=y4p[:],
                        scalar=bb[0:OUT_F, 4:5],
                        in1=s_sb[:, cs],
                        op0=mybir.AluOpType.add,
                        op1=mybir.AluOpType.mult,
                    )
                    x3_t[j] = None
                    if (j + 1) * TILE % OUT_CHUNK == 0:
                        ck = ((j + 1) * TILE) // OUT_CHUNK - 1
                        nc.sync.dma_start(
                            outd[ck],
                            out_sb[:, ck * OUT_CHUNK : (ck + 1) * OUT_CHUNK],
                        )

    nc.compile()
    return nc


_CACHED_NC = None


def kernel(h_v, h_w, e_vw, W1, b1, W2, b2, W3, b3, W4, b4):
    global LAST_RESULTS, _CACHED_NC

    h_w = np.asarray(h_w, np.float32)
    e_vw = np.asarray(e_vw, np.float32)
    W1 = np.asarray(W1, np.float32)
    W2 = np.asarray(W2, np.float32)
    W3 = np.asarray(W3, np.float32)
    W4 = np.asarray(W4, np.float32)
    b1 = np.asarray(b1, np.float32)
    b2 = np.asarray(b2, np.float32)
    b3 = np.asarray(b3, np.float32)
    b4 = np.asarray(b4, np.float32)

    # Host-side weight transform (exact reassociation of the reference math).
    W4s = W4.reshape(HID3, OUT_F, IN_F).sum(axis=2)
    b4s = b4.reshape(OUT_F, IN_F).sum(axis=1)
    s = h_w.reshape(-1)

    w3p = np.concatenate([W3[0:128], W3[128:256]], axis=1)  # [128, 256]
    bb = np.zeros((128, 5), np.float32)
    bb[:, 0] = b1
    bb[:, 1] = b2[0:128]
    bb[:, 2] = b2[128:256]
    bb[:, 3] = b3
    bb[0:OUT_F, 4] = b4s

    wpack = np.concatenate([W2, w3p, W4s, bb], axis=1)  # [128, 581]
    weights_map = {
        "w1d": np.ascontiguousarray(np.tile(W1, (4, 1)), NP_DT),
        "wpd": np.ascontiguousarray(wpack, np.float32),
    }

    in_maps = []
    for c in range(N_CORES):
        sl = slice(c * E_LOC, (c + 1) * E_LOC)
        e_loc = e_vw[sl]                       # [4096, 32]
        s_loc = s[sl]                          # [4096]
        # [128, 1024]: partition 32g+f holds feature f of edge group g
        e_t = np.ascontiguousarray(
            e_loc.T.reshape(EDGE_F, 4, E_LOC // 4)
            .transpose(1, 0, 2)
            .reshape(128, E_LOC // 4),
            NP_DT,
        )
        s_bcast = np.ascontiguousarray(
            np.broadcast_to(s_loc[None, :], (OUT_F, E_LOC)), np.float32
        )
        in_maps.append({"e_t": e_t, "s_b": s_bcast, **weights_map})

    if _CACHED_NC is None:
        _CACHED_NC = _build_bass(
            b2_halves_equal=bool(np.array_equal(b2[0:128], b2[128:256]))
        )
    nc = _CACHED_NC

    trace = bool(int(os.environ.get("KERNEL_TRACE", "0")))
    res = run_bass_kernel_spmd(
        nc, in_maps, core_ids=list(range(N_CORES)), trace=trace
    )
    LAST_RESULTS = res

    out = np.empty((E, OUT_F), np.float32)
    nck = E_LOC // OUT_CHUNK
    for c in range(N_CORES):
        o = res.results[c]["outd"]             # [nck, OUT_F, OUT_CHUNK]
        base = c * E_LOC
        for k in range(nck):
            out[base + k * OUT_CHUNK : base + (k + 1) * OUT_CHUNK] = o[k].T
    return out


# revision 44
# speedup vs baseline: 1.4058x; 1.0873x over previous
"""Trainium2 Bass kernel for nn_MessageFunction (gnn_message_passing).

Math (validated against the reference):
  The reference broadcasts h_w[:, :, None] -> (B*N, IN_F, N) and reshapes to
  [E, IN_F]; row-major order makes every row constant:
      h_w_rows[e, i] = h_w.reshape(-1)[e]   for all i.
  Hence the per-edge bmm collapses:
      m[e, o] = sum_i edge_output[e, o, i] * s[e]
              = s[e] * (x3[e] @ W4s[:, o] + b4s[o])
  with W4s = W4.reshape(HID3, OUT_F, IN_F).sum(-1), b4s = b4.reshape(OUT_F,
  IN_F).sum(-1), s = h_w.reshape(-1).  This is an exact reassociation (only
  f32 rounding differences) and removes the [E,128]@[128,4096] matmul + bmm.

Kernel: data-parallel over E = 32768 edges, 4096 per core across 8 cores,
MLP weights replicated, no cross-core communication.  Per core the MLP runs
features-on-partitions with edges streaming on the free dim:
    x1 = relu(W1.T @ eT)        K=32  -> [128, e]
    x2 = relu(W2.T @ x1)        K=128 -> [256, e] (two 128-part halves)
    x3 = relu(W3.T @ x2)        K=256 -> [128, e] (PSUM accumulation)
    y  = W4s.T @ x3             K=128 -> [64, e]  (col-packed 2 tiles/PSUM)
    out = (y + b4s) * s         one fused scalar_tensor_tensor on VectorE
Matmuls use float32r (full PE rate at N=512, near-fp32 precision).
"""

import os

import numpy as np

import concourse.bacc as bacc
import concourse.bass as bass
import concourse.mybir as mybir
import concourse.tile as tile
from concourse.bass_utils import run_bass_kernel_spmd

# Problem constants (hardcoded per the harness contract).
B, N = 8, 64
IN_F, OUT_F = 64, 64
EDGE_F = 32
HID1, HID2, HID3 = 128, 256, 128
E = B * N * N            # 32768
N_CORES = 8
E_LOC = E // N_CORES     # 4096
TILE = 512               # edges per tile (one PSUM bank per stage)
NT = E_LOC // TILE       # 8 tiles per core
OUT_CHUNK = 1024         # output DMA granularity (2 tiles)

F32 = mybir.dt.float32
# Matmul operand dtype: float32r streams at 1 cycle/row for N>=256 (same as
# bf16) with much better precision than bf16.
DT = mybir.dt.float32r
NP_DT = np.float32

# Module global: last BassKernelResults (test.py reads exec_time_ns from it).
LAST_RESULTS = None


def _build_bass(b2_halves_equal=True):
    nc = bacc.Bacc(
        "TRN2", target_bir_lowering=False, debug=False, num_devices=N_CORES
    )

    # Per-core inputs.  e_t is packed 4 edge-groups deep on partitions:
    # e_t[32*g + f, c] = e_vw[g*1024 + c, f] so one full-width DMA loads it.
    e_t = nc.dram_tensor("e_t", [128, E_LOC // 4], DT, kind="ExternalInput")
    s_b = nc.dram_tensor("s_b", [OUT_F, E_LOC], F32, kind="ExternalInput")
    # Replicated weights.  W1 is stacked 4x on partitions to serve the four
    # L1 row-tile positions.
    w1d = nc.dram_tensor("w1d", [128, HID1], DT, kind="ExternalInput")
    # Everything else packed into one DMA: [W2 | W3packed | W4s | biases]
    # = 256 + 256 + 64 + 5 columns.  Bias columns: b1, b2[:128], b2[128:],
    # b3, [b4s; pad].
    WPACK = HID2 + 2 * HID3 + OUT_F + 5
    wpd = nc.dram_tensor("wpd", [128, WPACK], DT, kind="ExternalInput")
    outd = nc.dram_tensor(
        "outd", [E_LOC // OUT_CHUNK, OUT_F, OUT_CHUNK], F32, kind="ExternalOutput"
    )

    # Relu pass engine schedule (per tile: L1, L2a, L2b, L3). 'A' = ScalarE,
    # 'V' = VectorE.  VectorE also runs the eight final bias+scale ops, so
    # ScalarE takes more of the 32 relu passes (20 A / 12 V).
    relu_sched = ["AVAV", "AVAA"] * (NT // 2)

    with tile.TileContext(nc) as tc:
        with (
            tc.tile_pool(name="wp", bufs=1) as wp,
            tc.tile_pool(name="io", bufs=4) as io,
            tc.tile_pool(name="acts", bufs=3) as acts,
            tc.tile_pool(name="ps", bufs=1, space="PSUM") as ps,
        ):
            e4 = wp.tile([128, E_LOC // 4], DT, tag="e4")
            w1 = wp.tile([128, HID1], DT, tag="w1")
            wpk = wp.tile([128, WPACK], DT, tag="wpk")
            s_sb = wp.tile([OUT_F, E_LOC], F32, tag="s_sb")
            out_sb = wp.tile([OUT_F, E_LOC], F32, tag="out_sb")
            # Views into the packed weight tile (bias columns bitcast to f32;
            # both dtypes are 4-byte so this is a pure reinterpret).
            w2 = wpk[:, 0:HID2]
            w3 = wpk[:, HID2 : HID2 + 2 * HID3]
            w4 = wpk[:, HID2 + 2 * HID3 : HID2 + 2 * HID3 + OUT_F]
            bb = wpk[:, HID2 + 2 * HID3 + OUT_F :].bitcast(F32)
            # Input loads on the two HWDGE rings (Sync + Scalar), which issue
            # in ~0.6us and stream FIFO; the GpSimd SWDGE path costs ~2us
            # fixed per transfer, far too slow for the startup path.
            # Sync ring carries ONLY what the first L1 matmul needs (ring is
            # FIFO end-to-end, so anything else here delays the whole kernel).
            nc.sync.dma_start(e4[:], e_t[:])
            nc.sync.dma_start(w1[:], w1d[:])
            nc.scalar.dma_start(wpk[:], wpd[:])
            nc.scalar.dma_start(s_sb[:], s_b[:])

            # PE warm-up: ~10 dependency-free matmuls on scratch data run
            # back-to-back during the input-load window, so the HAM clock
            # gate reaches 2.4 GHz before the first real matmul.  Garbage
            # values are fine — the scratch PSUM is never read.
            scratch = wp.tile([128, TILE], DT, tag="scratch")
            nc.gpsimd.memset(scratch[:].bitcast(F32), 1.0)

            def emit_dummies(n):
                for _ in range(n):
                    warm_ps = ps.tile([OUT_F, TILE], F32, tag="y4p", bufs=2)
                    nc.tensor.matmul(warm_ps[:], scratch[:, 0:OUT_F], scratch[:])

            emit_dummies(13)

            def relu_pass(dst, src, bias_col, eng):
                if eng == "A":
                    nc.scalar.activation(
                        dst, src, mybir.ActivationFunctionType.Relu, bias=bias_col
                    )
                else:
                    nc.vector.tensor_scalar(
                        out=dst,
                        in0=src,
                        scalar1=bias_col,
                        scalar2=0.0,
                        op0=mybir.AluOpType.add,
                        op1=mybir.AluOpType.max,
                    )

            # Software-pipelined emission, skewed so each pass result is
            # consumed one full iteration after it is produced — the PE
            # matmul stream never waits on a just-issued ScalarE/VectorE
            # pass.  Stage s of tile t runs in iteration t+s.
            x1_t = [None] * NT
            x2_t = [None] * NT
            x3_t = [None] * NT
            # P3 engine: 6 on ScalarE / 2 on VectorE (balance against the
            # fixed STT work on VectorE); P1 on VectorE, P2 on ScalarE.
            p3_eng = ["A", "A", "V", "A", "A", "A", "V", "A"]

            # Dummy matmuls bridging the pipeline-fill iterations, so the PE
            # never idles >3.4us (which would re-throttle the HAM clock gate).
            bridge = {}

            for i in range(NT + 4):
                if i in bridge:
                    emit_dummies(bridge[i])
                # S1 + P1 for tile i.  L1 is a K=32 row-tiled matmul: edge
                # group g = i//2 lives on partitions [32g, 32g+32) of e4 and
                # w1 (stacked), with the matching tile_position row.
                if 0 <= i < NT:
                    g = i // 2
                    gp = slice(32 * g, 32 * g + 32)
                    gc = slice((i % 2) * TILE, (i % 2) * TILE + TILE)
                    x1p = ps.tile([128, TILE], F32, tag="x1p", bufs=2)
                    nc.tensor.matmul(
                        x1p[:], w1[gp, :], e4[gp, gc], tile_position=(32 * g, 0)
                    )
                    x1 = acts.tile([128, TILE], DT, tag="x1")
                    relu_pass(x1[:], x1p[:], bb[:, 0:1], "V")
                    x1_t[i] = x1

                # S2 + P2 for tile i-1 (merged 2-bank PSUM, single pass)
                j = i - 1
                if 0 <= j < NT:
                    x2p = ps.tile([128, 2 * TILE], F32, tag="x2p", bufs=1)
                    nc.tensor.matmul(x2p[:, 0:TILE], w2[:, 0:128], x1_t[j][:])
                    nc.tensor.matmul(x2p[:, TILE : 2 * TILE], w2[:, 128:256], x1_t[j][:])
                    x2 = acts.tile([128, 2 * TILE], DT, tag="x2")
                    # A per-partition bias is constant along the free dim, so
                    # one merged pass is only valid when both b2 halves agree
                    # (always true for the zero biases here); otherwise fall
                    # back to two passes.
                    if b2_halves_equal:
                        nc.scalar.activation(
                            x2[:], x2p[:],
                            mybir.ActivationFunctionType.Relu, bias=bb[:, 1:2],
                        )
                    else:
                        nc.scalar.activation(
                            x2[:, 0:TILE], x2p[:, 0:TILE],
                            mybir.ActivationFunctionType.Relu, bias=bb[:, 1:2],
                        )
                        nc.scalar.activation(
                            x2[:, TILE : 2 * TILE], x2p[:, TILE : 2 * TILE],
                            mybir.ActivationFunctionType.Relu, bias=bb[:, 2:3],
                        )
                    x2_t[j] = x2
                    x1_t[j] = None

                # S3 + P3 for tile i-2
                j = i - 2
                if 0 <= j < NT:
                    x3ps = ps.tile([128, TILE], F32, tag="x3ps", bufs=2)
                    nc.tensor.matmul(
                        x3ps[:], w3[:, 0:128], x2_t[j][:, 0:TILE],
                        start=True, stop=False,
                    )
                    nc.tensor.matmul(
                        x3ps[:], w3[:, 128:256], x2_t[j][:, TILE : 2 * TILE],
                        start=False, stop=True,
                    )
                    x3 = acts.tile([128, TILE], DT, tag="x3")
                    relu_pass(x3[:], x3ps[:], bb[:, 3:4], p3_eng[j])
                    x3_t[j] = x3
                    x2_t[j] = None

                # S4 + P4 for tile i-3
                j = i - 3
                if 0 <= j < NT:
                    cs = slice(j * TILE, (j + 1) * TILE)
                    y4p = ps.tile([OUT_F, TILE], F32, tag="y4p", bufs=2)
                    nc.tensor.matmul(y4p[:], w4[:], x3_t[j][:])
                    nc.vector.scalar_tensor_tensor(
                        out=out_sb[:, cs],
                        in0=y4p[:],
                        scalar=bb[0:OUT_F, 4:5],
                        in1=s_sb[:, cs],
                        op0=mybir.AluOpType.add,
                        op1=mybir.AluOpType.mult,
                    )
                    x3_t[j] = None
                    if (j + 1) * TILE % OUT_CHUNK == 0:
                        ck = ((j + 1) * TILE) // OUT_CHUNK - 1
                        nc.sync.dma_start(
                            outd[ck],
                            out_sb[:, ck * OUT_CHUNK : (ck + 1) * OUT_CHUNK],
                        )

    nc.compile()
    return nc


_CACHED_NC = None


def kernel(h_v, h_w, e_vw, W1, b1, W2, b2, W3, b3, W4, b4):
    global LAST_RESULTS, _CACHED_NC

    h_w = np.asarray(h_w, np.float32)
    e_vw = np.asarray(e_vw, np.float32)
    W1 = np.asarray(W1, np.float32)
    W2 = np.asarray(W2, np.float32)
    W3 = np.asarray(W3, np.float32)
    W4 = np.asarray(W4, np.float32)
    b1 = np.asarray(b1, np.float32)
    b2 = np.asarray(b2, np.float32)
    b3 = np.asarray(b3, np.float32)
    b4 = np.asarray(b4, np.float32)

    # Host-side weight transform (exact reassociation of the reference math).
    W4s = W4.reshape(HID3, OUT_F, IN_F).sum(axis=2)
    b4s = b4.reshape(OUT_F, IN_F).sum(axis=1)
    s = h_w.reshape(-1)

    w3p = np.concatenate([W3[0:128], W3[128:256]], axis=1)  # [128, 256]
    bb = np.zeros((128, 5), np.float32)
    bb[:, 0] = b1
    bb[:, 1] = b2[0:128]
    bb[:, 2] = b2[128:256]
    bb[:, 3] = b3
    bb[0:OUT_F, 4] = b4s

    wpack = np.concatenate([W2, w3p, W4s, bb], axis=1)  # [128, 581]
    weights_map = {
        "w1d": np.ascontiguousarray(np.tile(W1, (4, 1)), NP_DT),
        "wpd": np.ascontiguousarray(wpack, np.float32),
    }

    in_maps = []
    for c in range(N_CORES):
        sl = slice(c * E_LOC, (c + 1) * E_LOC)
        e_loc = e_vw[sl]                       # [4096, 32]
        s_loc = s[sl]                          # [4096]
        # [128, 1024]: partition 32g+f holds feature f of edge group g
        e_t = np.ascontiguousarray(
            e_loc.T.reshape(EDGE_F, 4, E_LOC // 4)
            .transpose(1, 0, 2)
            .reshape(128, E_LOC // 4),
            NP_DT,
        )
        s_bcast = np.ascontiguousarray(
            np.broadcast_to(s_loc[None, :], (OUT_F, E_LOC)), np.float32
        )
        in_maps.append({"e_t": e_t, "s_b": s_bcast, **weights_map})

    if _CACHED_NC is None:
        _CACHED_NC = _build_bass(
            b2_halves_equal=bool(np.array_equal(b2[0:128], b2[128:256]))
        )
    nc = _CACHED_NC

    trace = bool(int(os.environ.get("KERNEL_TRACE", "0")))
    res = run_bass_kernel_spmd(
        nc, in_maps, core_ids=list(range(N_CORES)), trace=trace
    )
    LAST_RESULTS = res

    out = np.empty((E, OUT_F), np.float32)
    nck = E_LOC // OUT_CHUNK
    for c in range(N_CORES):
        o = res.results[c]["outd"]             # [nck, OUT_F, OUT_CHUNK]
        base = c * E_LOC
        for k in range(nck):
            out[base + k * OUT_CHUNK : base + (k + 1) * OUT_CHUNK] = o[k].T
    return out


# revision 57
# speedup vs baseline: 1.4273x; 1.0153x over previous
"""Trainium2 Bass kernel for nn_MessageFunction (gnn_message_passing).

Math (validated against the reference):
  The reference broadcasts h_w[:, :, None] -> (B*N, IN_F, N) and reshapes to
  [E, IN_F]; row-major order makes every row constant:
      h_w_rows[e, i] = h_w.reshape(-1)[e]   for all i.
  Hence the per-edge bmm collapses:
      m[e, o] = sum_i edge_output[e, o, i] * s[e]
              = s[e] * (x3[e] @ W4s[:, o] + b4s[o])
  with W4s = W4.reshape(HID3, OUT_F, IN_F).sum(-1), b4s = b4.reshape(OUT_F,
  IN_F).sum(-1), s = h_w.reshape(-1).  This is an exact reassociation (only
  f32 rounding differences) and removes the [E,128]@[128,4096] matmul + bmm.

Kernel: data-parallel over E = 32768 edges, 4096 per core across 8 cores,
MLP weights replicated, no cross-core communication.  Per core the MLP runs
features-on-partitions with edges streaming on the free dim:
    x1 = relu(W1.T @ eT)        K=32  -> [128, e]
    x2 = relu(W2.T @ x1)        K=128 -> [256, e] (two 128-part halves)
    x3 = relu(W3.T @ x2)        K=256 -> [128, e] (PSUM accumulation)
    y  = W4s.T @ x3             K=128 -> [64, e]  (col-packed 2 tiles/PSUM)
    out = (y + b4s) * s         one fused scalar_tensor_tensor on VectorE
Matmuls use float32r (full PE rate at N=512, near-fp32 precision).
"""

import os

import numpy as np

import concourse.bacc as bacc
import concourse.bass as bass
import concourse.mybir as mybir
import concourse.tile as tile
from concourse.bass_utils import run_bass_kernel_spmd

# Problem constants (hardcoded per the harness contract).
B, N = 8, 64
IN_F, OUT_F = 64, 64
EDGE_F = 32
HID1, HID2, HID3 = 128, 256, 128
E = B * N * N            # 32768
N_CORES = 8
E_LOC = E // N_CORES     # 4096
TILE = 512               # edges per tile (one PSUM bank per stage)
NT = E_LOC // TILE       # 8 tiles per core
OUT_CHUNK = 1024         # output DMA granularity (2 tiles)

F32 = mybir.dt.float32
# Matmul operand dtype: float32r streams at 1 cycle/row for N>=256 (same as
# bf16) with much better precision than bf16.
DT = mybir.dt.float32r
NP_DT = np.float32

# Module global: last BassKernelResults (test.py reads exec_time_ns from it).
LAST_RESULTS = None


def _build_bass(b2_halves_equal=True, b4_nonzero=False):
    nc = bacc.Bacc(
        "TRN2", target_bir_lowering=False, debug=False, num_devices=N_CORES
    )

    # Per-core inputs.  e_t is packed 4 edge-groups deep on partitions:
    # e_t[32*g + f, c] = e_vw[g*1024 + c, f] so one full-width DMA loads it.
    e_t = nc.dram_tensor("e_t", [128, E_LOC // 4], DT, kind="ExternalInput")
    # s pair-stacked: rows 0-63 broadcast s of even tiles, 64-127 odd tiles;
    # column 512*p + c maps to edges 1024p + c (rows<64) / 1024p + 512 + c.
    s_b = nc.dram_tensor("s_b", [128, E_LOC // 2], F32, kind="ExternalInput")
    # s duplicated on two partitions for the optional K=2 bias matmul.
    srd = nc.dram_tensor("srd", [2, E_LOC // 2], DT, kind="ExternalInput")
    # Replicated weights.  W1 is stacked 4x on partitions to serve the four
    # L1 row-tile positions.
    w1d = nc.dram_tensor("w1d", [128, HID1], DT, kind="ExternalInput")
    # Everything else packed into one DMA:
    # [W2 | W3packed | W4sA | W4sB | biases] = 256+256+128+128+5 cols.
    # W4sA = [W4s | 0], W4sB = [0 | W4s]: block-diagonal L4 so a pair of
    # tiles lands on disjoint PSUM partition halves of one bank.
    # Then a 128-col block whose rows 0-1 hold [b4s|0] / [0|b4s] (K=2 bias
    # matmul lhsT), then bias columns: b1, b2[:128], b2[128:], b3, pad.
    WPACK = HID2 + 2 * HID3 + 2 * 128 + 128 + 5
    wpd = nc.dram_tensor("wpd", [128, WPACK], DT, kind="ExternalInput")
    outd = nc.dram_tensor(
        "outd", [E_LOC // OUT_CHUNK, 128, OUT_CHUNK // 2], F32, kind="ExternalOutput"
    )

    with tile.TileContext(nc) as tc:
        with (
            tc.tile_pool(name="wp", bufs=1) as wp,
            tc.tile_pool(name="io", bufs=4) as io,
            tc.tile_pool(name="acts", bufs=3) as acts,
            tc.tile_pool(name="ps", bufs=1, space="PSUM") as ps,
        ):
            e4 = wp.tile([128, E_LOC // 4], DT, tag="e4")
            w1 = wp.tile([128, HID1], DT, tag="w1")
            wpk = wp.tile([128, WPACK], DT, tag="wpk")
            s_sb = wp.tile([128, E_LOC // 2], F32, tag="s_sb")
            sr2 = wp.tile([2, E_LOC // 2], DT, tag="sr2")
            out_sb = wp.tile([128, E_LOC // 2], F32, tag="out_sb")
            # Views into the packed weight tile (bias columns bitcast to f32;
            # both dtypes are 4-byte so this is a pure reinterpret).
            w2 = wpk[:, 0:HID2]
            w3 = wpk[:, HID2 : HID2 + 2 * HID3]
            w4a = wpk[:, HID2 + 2 * HID3 : HID2 + 2 * HID3 + 128]
            w4b = wpk[:, HID2 + 2 * HID3 + 128 : HID2 + 2 * HID3 + 256]
            b4mm = wpk[0:2, HID2 + 2 * HID3 + 256 : HID2 + 2 * HID3 + 384]
            bb = wpk[:, HID2 + 2 * HID3 + 384 :].bitcast(F32)
            # Input loads on the two HWDGE rings (Sync + Scalar), which issue
            # in ~0.6us and stream FIFO; the GpSimd SWDGE path costs ~2us
            # fixed per transfer, far too slow for the startup path.
            # Sync ring carries ONLY what the first L1 matmul needs (ring is
            # FIFO end-to-end, so anything else here delays the whole kernel).
            nc.sync.dma_start(e4[:], e_t[:])
            nc.sync.dma_start(w1[:], w1d[:])
            nc.scalar.dma_start(wpk[:], wpd[:])
            nc.scalar.dma_start(s_sb[:], s_b[:])
            if b4_nonzero:
                nc.scalar.dma_start(sr2[:], srd[:])

            # PE warm-up: ~10 dependency-free matmuls on scratch data run
            # back-to-back during the input-load window, so the HAM clock
            # gate reaches 2.4 GHz before the first real matmul.  Garbage
            # values are fine — the scratch PSUM is never read.
            scratch = wp.tile([128, TILE], DT, tag="scratch")
            nc.gpsimd.memset(scratch[:].bitcast(F32), 1.0)

            def emit_dummies(n):
                for _ in range(n):
                    warm_ps = ps.tile([128, TILE], F32, tag="y4p", bufs=2)
                    nc.tensor.matmul(warm_ps[:], scratch[:, 0:128], scratch[:])

            emit_dummies(13)

            def relu_pass(dst, src, bias_col, eng):
                if eng == "A":
                    nc.scalar.activation(
                        dst, src, mybir.ActivationFunctionType.Relu, bias=bias_col
                    )
                else:
                    nc.vector.tensor_scalar(
                        out=dst,
                        in0=src,
                        scalar1=bias_col,
                        scalar2=0.0,
                        op0=mybir.AluOpType.add,
                        op1=mybir.AluOpType.max,
                    )

            # Software-pipelined emission, skewed so each pass result is
            # consumed one full iteration after it is produced — the PE
            # matmul stream never waits on a just-issued ScalarE/VectorE
            # pass.  Stage s of tile t runs in iteration t+s.
            x1_t = [None] * NT
            x2_t = [None] * NT
            x3_t = [None] * NT
            # P3 engine: 6 on ScalarE / 2 on VectorE (balance against the
            # fixed STT work on VectorE); P1 on VectorE, P2 on ScalarE.
            p3_eng = ["A", "A", "V", "A", "A", "A", "V", "A"]

            # Dummy matmuls bridging the pipeline-fill iterations, so the PE
            # never idles >3.4us (which would re-throttle the HAM clock gate).
            bridge = {}

            for i in range(NT + 4):
                if i in bridge:
                    emit_dummies(bridge[i])
                # S1 + P1 for tile i.  L1 is a K=32 row-tiled matmul: edge
                # group g = i//2 lives on partitions [32g, 32g+32) of e4 and
                # w1 (stacked), with the matching tile_position row.
                if 0 <= i < NT:
                    g = i // 2
                    gp = slice(32 * g, 32 * g + 32)
                    gc = slice((i % 2) * TILE, (i % 2) * TILE + TILE)
                    x1p = ps.tile([128, TILE], F32, tag="x1p", bufs=2)
                    nc.tensor.matmul(
                        x1p[:], w1[gp, :], e4[gp, gc], tile_position=(32 * g, 0)
                    )
                    x1 = acts.tile([128, TILE], DT, tag="x1")
                    relu_pass(x1[:], x1p[:], bb[:, 0:1], "V")
                    x1_t[i] = x1

                # S2 + P2 for tile i-1 (merged 2-bank PSUM, single pass)
                j = i - 1
                if 0 <= j < NT:
                    x2p = ps.tile([128, 2 * TILE], F32, tag="x2p", bufs=1)
                    nc.tensor.matmul(x2p[:, 0:TILE], w2[:, 0:128], x1_t[j][:])
                    nc.tensor.matmul(x2p[:, TILE : 2 * TILE], w2[:, 128:256], x1_t[j][:])
                    x2 = acts.tile([128, 2 * TILE], DT, tag="x2")
                    # A per-partition bias is constant along the free dim, so
                    # one merged pass is only valid when both b2 halves agree
                    # (always true for the zero biases here); otherwise fall
                    # back to two passes.
                    if b2_halves_equal:
                        nc.scalar.activation(
                            x2[:], x2p[:],
                            mybir.ActivationFunctionType.Relu, bias=bb[:, 1:2],
                        )
                    else:
                        nc.scalar.activation(
                            x2[:, 0:TILE], x2p[:, 0:TILE],
                            mybir.ActivationFunctionType.Relu, bias=bb[:, 1:2],
                        )
                        nc.scalar.activation(
                            x2[:, TILE : 2 * TILE], x2p[:, TILE : 2 * TILE],
                            mybir.ActivationFunctionType.Relu, bias=bb[:, 2:3],
                        )
                    x2_t[j] = x2
                    x1_t[j] = None

                # S3 + P3 for tile i-2
                j = i - 2
                if 0 <= j < NT:
                    x3ps = ps.tile([128, TILE], F32, tag="x3ps", bufs=2)
                    nc.tensor.matmul(
                        x3ps[:], w3[:, 0:128], x2_t[j][:, 0:TILE],
                        start=True, stop=False,
                    )
                    nc.tensor.matmul(
                        x3ps[:], w3[:, 128:256], x2_t[j][:, TILE : 2 * TILE],
                        start=False, stop=True,
                    )
                    x3 = acts.tile([128, TILE], DT, tag="x3")
                    relu_pass(x3[:], x3ps[:], bb[:, 3:4], p3_eng[j])
                    x3_t[j] = x3
                    x2_t[j] = None

                # S4 + P4 for the tile PAIR ending at tile i-2 (odd tiles).
                # Block-diagonal weights put pair tiles (2p, 2p+1) on PSUM
                # partition halves [0:64) / [64:128) of one bank; one
                # full-width tensor_mul applies the per-edge h_w scale.
                j = i - 2
                if 0 <= j < NT and j % 2 == 1:
                    p = j // 2
                    cs = slice(p * TILE, (p + 1) * TILE)
                    y4p = ps.tile([128, TILE], F32, tag="y4p", bufs=2)
                    nc.tensor.matmul(
                        y4p[:], w4a[:], x3_t[j - 1][:], start=True, stop=False
                    )
                    nc.tensor.matmul(
                        y4p[:], w4b[:], x3_t[j][:],
                        start=False, stop=not b4_nonzero,
                    )
                    if b4_nonzero:
                        # += b4s[o] * s[e] per block, via a K=2 matmul:
                        # lhsT rows = [b4s|0], [0|b4s]; rhs rows = s even/odd.
                        nc.tensor.matmul(
                            y4p[:], b4mm, sr2[:, cs], start=False, stop=True
                        )
                    nc.vector.tensor_mul(out_sb[:, cs], y4p[:], s_sb[:, cs])
                    x3_t[j - 1] = None
                    x3_t[j] = None
                    if (p + 1) * TILE % (OUT_CHUNK // 2) == 0:
                        ck = ((p + 1) * TILE) // (OUT_CHUNK // 2) - 1
                        w = OUT_CHUNK // 2
                        nc.sync.dma_start(
                            outd[ck], out_sb[:, ck * w : (ck + 1) * w]
                        )

    nc.compile()
    return nc


_CACHED_NC = None


def kernel(h_v, h_w, e_vw, W1, b1, W2, b2, W3, b3, W4, b4):
    global LAST_RESULTS, _CACHED_NC

    h_w = np.asarray(h_w, np.float32)
    e_vw = np.asarray(e_vw, np.float32)
    W1 = np.asarray(W1, np.float32)
    W2 = np.asarray(W2, np.float32)
    W3 = np.asarray(W3, np.float32)
    W4 = np.asarray(W4, np.float32)
    b1 = np.asarray(b1, np.float32)
    b2 = np.asarray(b2, np.float32)
    b3 = np.asarray(b3, np.float32)
    b4 = np.asarray(b4, np.float32)

    # Host-side weight transform (exact reassociation of the reference math).
    W4s = W4.reshape(HID3, OUT_F, IN_F).sum(axis=2)
    b4s = b4.reshape(OUT_F, IN_F).sum(axis=1)
    s = h_w.reshape(-1)

    w3p = np.concatenate([W3[0:128], W3[128:256]], axis=1)  # [128, 256]
    bb = np.zeros((128, 5), np.float32)
    bb[:, 0] = b1
    bb[:, 1] = b2[0:128]
    bb[:, 2] = b2[128:256]
    bb[:, 3] = b3
    bb[0:OUT_F, 4] = b4s

    w4A = np.concatenate([W4s, np.zeros((HID3, 64), np.float32)], axis=1)
    w4B = np.concatenate([np.zeros((HID3, 64), np.float32), W4s], axis=1)
    b4blk = np.zeros((128, 128), np.float32)
    b4blk[0, 0:64] = b4s
    b4blk[1, 64:128] = b4s
    wpack = np.concatenate([W2, w3p, w4A, w4B, b4blk, bb], axis=1)
    weights_map = {
        "w1d": np.ascontiguousarray(np.tile(W1, (4, 1)), NP_DT),
        "wpd": np.ascontiguousarray(wpack, np.float32),
    }

    in_maps = []
    for c in range(N_CORES):
        sl = slice(c * E_LOC, (c + 1) * E_LOC)
        e_loc = e_vw[sl]                       # [4096, 32]
        s_loc = s[sl]                          # [4096]
        # [128, 1024]: partition 32g+f holds feature f of edge group g
        e_t = np.ascontiguousarray(
            e_loc.T.reshape(EDGE_F, 4, E_LOC // 4)
            .transpose(1, 0, 2)
            .reshape(128, E_LOC // 4),
            NP_DT,
        )
        # pair-stacked s broadcast: [128, 2048]
        s_pairs = s_loc.reshape(NT // 2, 2, TILE)           # [pair, half, 512]
        s_bcast = np.empty((128, E_LOC // 2), np.float32)
        s_bcast[0:64] = np.repeat(s_pairs[:, 0, :], 1, axis=0).reshape(
            NT // 2 * TILE
        )[None, :]
        s_bcast[64:128] = s_pairs[:, 1, :].reshape(NT // 2 * TILE)[None, :]
        sr2 = np.ascontiguousarray(
            np.stack([s_bcast[0], s_bcast[64]]), NP_DT
        )  # [2, 2048]
        in_maps.append(
            {"e_t": e_t, "s_b": s_bcast, "srd": sr2, **weights_map}
        )

    if _CACHED_NC is None:
        _CACHED_NC = _build_bass(
            b2_halves_equal=bool(np.array_equal(b2[0:128], b2[128:256])),
            b4_nonzero=bool(np.any(b4s != 0.0)),
        )
    nc = _CACHED_NC

    trace = bool(int(os.environ.get("KERNEL_TRACE", "0")))
    res = run_bass_kernel_spmd(
        nc, in_maps, core_ids=list(range(N_CORES)), trace=trace
    )
    LAST_RESULTS = res

    out = np.empty((E, OUT_F), np.float32)
    for c in range(N_CORES):
        o = res.results[c]["outd"]             # [4, 128, 512]: pair chunks
        base = c * E_LOC
        for p in range(NT // 2):
            out[base + 2 * p * TILE : base + (2 * p + 1) * TILE] = o[p, 0:64].T
            out[base + (2 * p + 1) * TILE : base + (2 * p + 2) * TILE] = (
                o[p, 64:128].T
            )
    return out
